# revision 48
# baseline (speedup 1.0000x reference)
"""Trainium2 Bass kernel for nn_MA_73478300500338 (retrieval_knn).

Pipeline (reference semantics):
  q = relu(query_embedding)                      [B, D]
  sim = cos(q, memory_keys); idx = top_k(sim, 32)
  mk = memory_keys[idx]
  qt = relu(q @ Wq + bq); mt = relu(mk @ Wm + bm)
  attended = sum_j mt[:, j, :]   (softmax over size-1 axis == 1)
  ma = LN(attended + qt) * gamma + beta
  out = [q, ma] @ Wc + bc                        [B, C]

Distribution (8 NeuronCores):
  Phase 1 (bf16 PE): memory bank sharded 8x (12500 rows/core). Each core
    computes bf16 dot products q . (k/|k|) for its shard (fp32 accumulate)
    and emits the max over each 10-wide block ("block-maxima", fp32) via DVE
    tensor_reduce straight out of PSUM. No top-k / index pass on device.
  Host: merges 8x2500 block maxima per query, takes the top R=64 blocks
    (provably a superset of every block containing a true top-32 key, given
    |bf16 dot err| <= eps; measured max err 1.0e-2 on this dataset, margin
    2*eps = 3e-2 used, max blocks needed 46), rescores those blocks exactly
    in fp64 and picks the exact global top-32. Exactness argument: the 32nd
    largest block-max T satisfies v32 >= T - eps (32 distinct keys sit at
    their block maxima), and every member's block-max >= v32 - eps, so all
    member blocks lie in {bmax >= T - 2*eps}. A count check + exact full
    fallback guards the fixed R.
  Phase 2 (bf16 PE): queries sharded 8x (32/core). Attention MLP, transpose-
    free layernorm (cross-partition moments via ones-matmul, gamma/beta
    applied through a rank-2 broadcast matmul), output projection.
"""

import os
import sys
import json

import numpy as np
import ml_dtypes

os.environ.setdefault("MYCRO_LOCAL_CACHE", "1")
if "/opt/trn_rl_repo" not in sys.path:
    sys.path.insert(0, "/opt/trn_rl_repo")

try:
    import jax as _jax
    _jax.config.update("jax_compilation_cache_dir", "/tmp/jax_cache_nn_ma")
    _jax.config.update("jax_persistent_cache_min_entry_size_bytes", -1)
    _jax.config.update("jax_persistent_cache_min_compile_time_secs", 0.5)
except Exception:
    pass

import bass_rust
import concourse.bass as bass
import concourse.bacc as bacc
import concourse.mybir as mybir
import concourse.tile as tile
from concourse.vector_clock import ScopedClock

# ---------------------------------------------------------------------------
# Workaround: this walrus build supports a single sync-wait per CTRL
# instruction, but Tile's stock tail drain carries one wait per busy
# processor. Split them into standalone single-wait instructions.
# ---------------------------------------------------------------------------


def _patched_drain_and_barrier(self, tick_clock, wait_clock):
    nc = self.nc
    with nc.discard():
        probe = nc.sync.drain()
        wait_clock.add_sem_waits(
            probe.ins, ScopedClock({None: tick_clock.global_clock})
        )
        j = json.loads(nc.instruction_to_json(probe.ins))
    waits = (j.get("sync_info") or {}).get("on_wait") or []
    for w in waits:
        sem = bass_rust.SemaphoreHandle(w["ant_name"], w["id"])
        assert w["wait_mode"] == "sem-ge-imm", w
        nc.sync.wait_ge(sem, w["wait_value"])
    nc.sync.drain()
    nc.all_engine_barrier()
    popped = nc._tile_sem_poison_stack.pop()
    assert popped is self._sem_poison
    nc.clear_and_free_semaphores(list(self.sems.allocated().values()))
    nc.all_engine_barrier()


tile.TileContext._drain_and_barrier = _patched_drain_and_barrier

# ---------------------------------------------------------------------------
# Problem shapes (hardcoded per spec)
# ---------------------------------------------------------------------------
B, N, D = 256, 100000, 512
AU, C, K = 256, 100, 32
NCORES = 8
SH = N // NCORES          # 12500 keys per core
W = 500                   # dot-product window (one PSUM-bank-pair round)
NW = SH // W              # 25 windows per core
BLK = 5                   # block-max granularity
WB = W // BLK             # 100 blocks per window
NBLK = SH // BLK          # 2500 blocks per core
DC = D // 128             # 4 contraction chunks
KSCALE = 32.0             # fp8 prescale on normalized keys (subnormal dodge)
EPS_LN = 1e-5
# ~2x the measured max |fp8 dot - exact| (0.168) incl. bf16 max rounding
MARGIN = 3.5e-1
RTOP = 320                # blocks rescored per query (max needed ~240)

F32 = mybir.dt.float32
BF16 = mybir.dt.bfloat16
F8 = mybir.dt.float8e4

BF = ml_dtypes.bfloat16
NP8 = mybir.dt.np(F8)

# phase-1 reduce paths: P1_ASET windows use DVE tensor_reduce on the
# original column order; all others are host-permuted to block-major and
# max-accumulated via 2x-mode bf16 tensor_max (Act copy + DVE slabs).
P1_ASET = frozenset({1, 6, 11, 16, 18, 21, 24})
P1_RUNS = [[0], [2, 3, 4, 5], [7, 8, 9, 10], [12, 13, 14, 15],
           [17], [19, 20], [22, 23]]


def _p1_colperm():
    """Device column p reads host column perm[p]; slab windows go
    block-major: device pos j*WB+nb <- original col nb*BLK+j."""
    perm = np.arange(SH).reshape(NW, W)
    slab = np.arange(W).reshape(WB, BLK).T.reshape(W)
    for w in range(NW):
        if w not in P1_ASET:
            perm[w] = w * W + slab
    return perm.reshape(SH)

_cache = {}


# ---------------------------------------------------------------------------
# Phase 1: bf16 dots + 10-wide block maxima
# ---------------------------------------------------------------------------


def _build_phase1():
    nc = bacc.Bacc()
    qT = nc.dram_tensor("qT", [D, B], F8, kind="ExternalInput")
    keysTn = nc.dram_tensor("keysTn", [D, SH], F8, kind="ExternalInput")
    bmax = nc.dram_tensor("bmax", [2, 128, NBLK], BF16, kind="ExternalOutput")

    # window groups per DMA: small at the front for a fast pipeline start
    GROUPS = [1, 2, 2, 4, 4, 4, 4, 4]
    assert sum(GROUPS) == NW
    OSPLITS = [11, 16, 21, 25]  # chunk boundaries aligned to slab runs

    with tile.TileContext(nc) as tc:
        with (
            tc.tile_pool(name="persist", bufs=1) as persist,
            tc.tile_pool(name="keys", bufs=3) as keysp,
            tc.tile_pool(name="cw", bufs=2) as cwp,
            tc.tile_pool(name="psum", bufs=4, space="PSUM") as psump,
        ):
            # PE warm-up: the pstate ramp needs ~3us of continuous busy to
            # reach full clock; burn the DMA lead-in on dummy matmuls.
            # The scratch PSUM comes from the ps pool (recycled later).
            scr = persist.tile([128, 256], F8)
            nc.gpsimd.memset(scr, 0.25)
            # pin the Act function table (Copy) during the lead-in too
            pint = persist.tile([1, 2], F32)
            nc.vector.memset(pint, 0.0)
            nc.scalar.copy(out=pint[:, 1:2], in_=pint[:, 0:1])
            wps = psump.tile([128, 2, 512], F32, tag="ps")
            for i in range(10):
                nc.tensor.matmul(
                    wps[0:16, 0, 0:256], scr[:, 0:16], scr,
                    start=True, stop=True,
                )

            qr = persist.tile([128, DC, B], F8)
            nc.sync.dma_start(
                out=qr,
                in_=bass.AP(qT, 0, [[B, 128], [128 * B, DC], [1, B]]),
            )
            bs = persist.tile([128, 2, NBLK], BF16)

            emitted = set()          # windows whose bs write has been emitted
            flushed = [False] * len(OSPLITS)
            pending = []             # deferred slab-group emissions

            def emit_group(cw_w0, rl, cw):
                bs_g = bs[
                    :, :, cw_w0 * WB:(cw_w0 + rl) * WB
                ].rearrange("p b (ws nb) -> p b ws nb", nb=WB)
                nc.vector.tensor_max(
                    out=bs_g,
                    in0=cw[:, :, :, 0:WB],
                    in1=cw[:, :, :, WB:2 * WB],
                )
                for j in range(2, BLK):
                    nc.vector.tensor_max(
                        out=bs_g, in0=bs_g,
                        in1=cw[:, :, :, j * WB:(j + 1) * WB],
                    )
                emitted.update(range(cw_w0, cw_w0 + rl))

            def try_flush():
                for i, osp in enumerate(OSPLITS):
                    if not flushed[i] and all(x in emitted for x in range(osp)):
                        flushed[i] = True
                        lo = (OSPLITS[i - 1] if i else 0) * WB
                        nc.gpsimd.dma_start(
                            out=bass.AP(
                                bmax, lo,
                                [[NBLK, 128], [128 * NBLK, 2],
                                 [1, osp * WB - lo]],
                            ),
                            in_=bs[:, :, lo:osp * WB],
                        )

            w0 = 0
            for gw in GROUPS:
                kt = keysp.tile([128, DC, gw * W], F8, tag="kt")
                nc.sync.dma_start(
                    out=kt,
                    in_=bass.AP(
                        keysTn,
                        w0 * W,
                        [[SH, 128], [128 * SH, DC], [1, gw * W]],
                    ),
                )
                for wi in range(gw):
                    w = w0 + wi
                    ps = psump.tile([128, 2, 512], F32, tag="ps")
                    for bc in range(2):
                        for c2 in range(2):
                            # fp8 DoubleRow: one pass contracts 256 rows
                            nc.tensor.matmul(
                                ps[:, bc, 0:W],
                                qr[:, 2 * c2:2 * c2 + 2,
                                   bc * 128:(bc + 1) * 128],
                                kt[:, 2 * c2:2 * c2 + 2,
                                   wi * W:(wi + 1) * W],
                                start=(c2 == 0),
                                stop=(c2 == 1),
                                perf_mode=mybir.MatmulPerfMode.DoubleRow,
                            )
                    if w in P1_ASET:
                        # direct path: DVE block-max straight from PSUM;
                        # emit before any pending slab group (it is ready
                        # earlier, and DVE executes in-order)
                        nc.vector.tensor_reduce(
                            out=bs[:, :, w * WB:(w + 1) * WB],
                            in_=ps[:, :, 0:W].rearrange(
                                "p b (nb k) -> p b nb k", k=BLK
                            ),
                            axis=mybir.AxisListType.X,
                            op=mybir.AluOpType.max,
                        )
                        emitted.add(w)
                        # pop all but the newest pending group: its gating
                        # Act copy just finished queueing, so running it now
                        # would head-block later (already-ready) reduces
                        while len(pending) > 1:
                            emit_group(*pending.pop(0))
                    else:
                        # slab path (host permuted these windows to block-
                        # major): Act copies PSUM->SBUF bf16 per window; DVE
                        # max-accumulates the run's 5 packed slabs in 2x
                        # mode, deferred past the next direct reduce
                        run = next(r for r in P1_RUNS if w in r)
                        slot = run.index(w)
                        rl = len(run)
                        if slot == 0:
                            cw = cwp.tile([128, 2, rl, 512], BF16, tag=f"cw{rl}")
                            cw_w0 = w
                        nc.scalar.copy(
                            out=cw[:, :, slot, 0:W], in_=ps[:, :, 0:W]
                        )
                        if slot == rl - 1:
                            pending.append((cw_w0, rl, cw))
                    try_flush()
                w0 += gw
            while pending:
                emit_group(*pending.pop(0))
            try_flush()
    nc.finalize()
    return nc


# ---------------------------------------------------------------------------
# Phase 2: attention MLP + LN + output projection (32 queries per core)
# ---------------------------------------------------------------------------
BQ = B // NCORES          # 32 queries per core
NK = BQ * K               # 1024 gathered key columns per core
AC = AU // 128            # 2 au chunks
WCC = (D + AU) // 128     # 6 Wc contraction chunks


def _build_phase2():
    nc = bacc.Bacc()
    qTc = nc.dram_tensor("qTc", [D, BQ], BF16, kind="ExternalInput")
    mkT = nc.dram_tensor("mkT", [D, NK], BF16, kind="ExternalInput")
    Wq = nc.dram_tensor("Wq", [D, AU], BF16, kind="ExternalInput")
    Wm = nc.dram_tensor("Wm", [D, AU], BF16, kind="ExternalInput")
    Wc = nc.dram_tensor("Wc", [D + AU, C], BF16, kind="ExternalInput")
    bq = nc.dram_tensor("bq", [AU], F32, kind="ExternalInput")
    bm = nc.dram_tensor("bm", [AU], F32, kind="ExternalInput")
    # rows: gamma, beta, -gamma
    gbT = nc.dram_tensor("gbT", [3, AU], F32, kind="ExternalInput")
    bc_ = nc.dram_tensor("bc_", [C], F32, kind="ExternalInput")
    out = nc.dram_tensor("out", [BQ, C], F32, kind="ExternalOutput")

    with tile.TileContext(nc) as tc:
        with (
            tc.tile_pool(name="p", bufs=1) as pool,
            tc.tile_pool(name="psum", bufs=2, space="PSUM") as psump,
            tc.tile_pool(name="psum1", bufs=1, space="PSUM") as psump1,
            tc.tile_pool(name="psumo", bufs=1, space="PSUM") as psumo,
            tc.tile_pool(name="warm", bufs=1, space="PSUM") as warmp,
        ):
            # PE warm-up during the DMA lead-in (pstate ramp)
            scr = pool.tile([128, 256], BF16)
            nc.gpsimd.memset(scr, 0.25)
            wps = warmp.tile([16, 256], F32)
            for i in range(10):
                nc.tensor.matmul(
                    wps, scr[:, 0:16], scr, start=True, stop=True,
                )
            # pin the activation table to the set holding relu+square+sqrt
            # +copy so no mid-kernel table reload is needed
            sqscr = pool.tile([1, 1], F32)
            nc.vector.memset(sqscr, 1.0)
            nc.scalar.sqrt(out=sqscr, in_=sqscr)

            # ---- loads: tiny constants via the Pool queue (SWDGE; keeps
            # the SP/HWDGE path clear for the big loads) ----
            bmc = pool.tile([128, AC], F32)
            nc.gpsimd.dma_start(out=bmc, in_=bass.AP(bm, 0, [[1, 128], [128, AC]]))
            bqc = pool.tile([128, AC], F32)
            nc.gpsimd.dma_start(out=bqc, in_=bass.AP(bq, 0, [[1, 128], [128, AC]]))

            wm = pool.tile([128, DC, AU], BF16)
            nc.sync.dma_start(
                out=wm, in_=bass.AP(Wm, 0, [[AU, 128], [128 * AU, DC], [1, AU]])
            )
            # mk h0 split by contraction pairs so the first matmul can
            # start after ~1/4 of the mk bytes have landed
            mk = pool.tile([128, DC, NK], BF16)
            for c2 in range(2):
                nc.sync.dma_start(
                    out=mk[:, 2 * c2:2 * c2 + 2, 0:512],
                    in_=bass.AP(
                        mkT, 2 * c2 * 128 * NK,
                        [[NK, 128], [128 * NK, 2], [1, 512]]
                    ),
                )
            nc.sync.dma_start(
                out=mk[:, :, 512:1024],
                in_=bass.AP(
                    mkT, 512, [[NK, 128], [128 * NK, DC], [1, 512]]
                ),
            )
            qr = pool.tile([128, DC, BQ], BF16)
            nc.sync.dma_start(
                out=qr, in_=bass.AP(qTc, 0, [[BQ, 128], [128 * BQ, DC], [1, BQ]])
            )
            wq = pool.tile([128, DC, AU], BF16)
            nc.sync.dma_start(
                out=wq, in_=bass.AP(Wq, 0, [[AU, 128], [128 * AU, DC], [1, AU]])
            )
            wc = pool.tile([128, WCC, C], BF16)
            nc.sync.dma_start(
                out=wc, in_=bass.AP(Wc, 0, [[C, 128], [128 * C, WCC], [1, C]])
            )
            # gamma/beta/-gamma as partition-0 rows (needed late; Pool queue)
            gb = pool.tile([1, 3, AC, 128], F32)
            nc.gpsimd.dma_start(
                out=gb, in_=bass.AP(gbT, 0, [[0, 1], [AU, 3], [128, AC], [1, 128]])
            )
            # bc broadcast rows [BQ, C]
            bcrow = pool.tile([BQ, C], F32)
            nc.gpsimd.dma_start(out=bcrow, in_=bass.AP(bc_, 0, [[0, BQ], [1, C]]))


            onesc = pool.tile([128, 1], F32)
            nc.vector.memset(onesc, 1.0)

            # ---- mtT = relu(Wm^T mk + bm): [AU, NK] f32 ----
            mtT = pool.tile([128, AC, NK], F32)
            for a in range(AC):
                for h in range(2):
                    ps = psump.tile([128, 512], F32, tag="ps")
                    for c in range(DC):
                        nc.tensor.matmul(
                            ps,
                            wm[:, c, a * 128:(a + 1) * 128],
                            mk[:, c, h * 512:(h + 1) * 512],
                            start=(c == 0),
                            stop=(c == DC - 1),
                        )
                    nc.scalar.activation(
                        out=mtT[:, a, h * 512:(h + 1) * 512],
                        in_=ps,
                        func=mybir.ActivationFunctionType.Relu,
                        bias=bmc[:, a:a + 1],
                        scale=1.0,
                    )

            # ---- xT = attendedT + qtT: [AU, BQ] f32 ----
            # attended reduce split per mk half: cols h*512.. hold queries
            # h*16.., so each half-reduce lands in its own query slice
            xT = pool.tile([128, AC, BQ], F32)
            attT = pool.tile([128, AC, BQ], F32)
            for a in range(AC):
                for h in range(2):
                    nc.vector.tensor_reduce(
                        out=attT[:, a, h * 16:(h + 1) * 16],
                        in_=mtT[:, a, h * 512:(h + 1) * 512].rearrange(
                            "p (b j) -> p b j", j=K
                        ),
                        axis=mybir.AxisListType.X,
                        op=mybir.AluOpType.add,
                    )
                ps = psump.tile([128, BQ], F32, tag="psq")
                for c in range(DC):
                    nc.tensor.matmul(
                        ps,
                        wq[:, c, a * 128:(a + 1) * 128],
                        qr[:, c, :],
                        start=(c == 0),
                        stop=(c == DC - 1),
                    )
                qt_a = pool.tile([128, BQ], F32, tag=f"qt{a}")
                nc.scalar.activation(
                    out=qt_a,
                    in_=ps,
                    func=mybir.ActivationFunctionType.Relu,
                    bias=bqc[:, a:a + 1],
                    scale=1.0,
                )
                nc.vector.tensor_add(out=xT[:, a, :], in0=attT[:, a, :], in1=qt_a)

            # ---- layernorm, transpose-free ----
            # moments via ones-matmul: [1, 2, BQ] = [sum(x); sum(x^2)]
            xsq = pool.tile([128, AC, BQ], F32)
            nc.scalar.activation(
                out=xsq, in_=xT, func=mybir.ActivationFunctionType.Square
            )
            psm = psump1.tile([1, 2, BQ], F32, tag="psm")
            for a in range(AC):
                nc.tensor.matmul(
                    psm[:, 0, :], onesc, xT[:, a, :],
                    start=(a == 0), stop=(a == AC - 1),
                )
            for a in range(AC):
                nc.tensor.matmul(
                    psm[:, 1, :], onesc, xsq[:, a, :],
                    start=(a == 0), stop=(a == AC - 1),
                )
            # rows [1, 8, BQ]: 0=mu 1=ex2 2=mu^2 3=var 4=std 5=rstd 6=mu*rstd
            rows = pool.tile([1, 8, BQ], F32)
            nc.scalar.mul(out=rows[:, 0:2, :], in_=psm, mul=1.0 / AU)
            nc.vector.tensor_mul(
                out=rows[:, 2, :], in0=rows[:, 0, :], in1=rows[:, 0, :]
            )
            nc.vector.tensor_sub(
                out=rows[:, 3, :], in0=rows[:, 1, :], in1=rows[:, 2, :]
            )
            eps = pool.tile([1, 1], F32)
            nc.vector.memset(eps, EPS_LN)
            nc.scalar.activation(
                out=rows[:, 4, :], in_=rows[:, 3, :],
                func=mybir.ActivationFunctionType.Sqrt,
                bias=eps, scale=1.0,
            )
            nc.vector.reciprocal(out=rows[:, 5, :], in_=rows[:, 4, :])
            nc.vector.tensor_mul(
                out=rows[:, 6, :], in0=rows[:, 0, :], in1=rows[:, 5, :]
            )
            ones_row = pool.tile([1, BQ], F32)
            nc.vector.memset(ones_row, 1.0)

            # C1 = gamma x rstd  (PSUM [128, AC, BQ]);
            # C0 = (-gamma) x (mu*rstd) + beta x ones
            psC = psump1.tile([128, 2, AC, BQ], F32, tag="psC")
            for a in range(AC):
                nc.tensor.matmul(
                    psC[:, 0, a, :], gb[:, 0, a, :], rows[:, 5, :],
                    start=True, stop=True,
                )
                nc.tensor.matmul(
                    psC[:, 1, a, :], gb[:, 2, a, :], rows[:, 6, :],
                    start=True, stop=False,
                )
                nc.tensor.matmul(
                    psC[:, 1, a, :], gb[:, 1, a, :], ones_row,
                    start=False, stop=True,
                )
            # maT = xT * C1 + C0, bf16
            maT = pool.tile([128, AC, BQ], BF16)
            tmp = pool.tile([128, AC, BQ], F32)
            for a in range(AC):
                nc.vector.tensor_mul(
                    out=tmp[:, a, :], in0=xT[:, a, :], in1=psC[:, 0, a, :]
                )
                nc.vector.tensor_add(
                    out=maT[:, a, :], in0=tmp[:, a, :], in1=psC[:, 1, a, :]
                )

            # ---- out = [q, ma] @ Wc + bc ----
            pso = psumo.tile([BQ, C], F32, tag="pso")
            for c in range(DC):
                nc.tensor.matmul(
                    pso, qr[:, c, :], wc[:, c, :],
                    start=(c == 0), stop=False,
                )
            for a in range(AC):
                nc.tensor.matmul(
                    pso, maT[:, a, :], wc[:, DC + a, :],
                    start=False, stop=(a == AC - 1),
                )
            ot = pool.tile([BQ, C], F32)
            nc.vector.tensor_add(out=ot, in0=bcrow, in1=pso)
            nc.sync.dma_start(out=out[:, :], in_=ot)
    nc.finalize()
    return nc


# ---------------------------------------------------------------------------
# SPMD runner with a persistent jitted executable
# ---------------------------------------------------------------------------


class _SpmdRunner:
    def __init__(self, nc, n_cores=NCORES):
        import jax
        from jax.sharding import Mesh, PartitionSpec
        from concourse import bass2jax
        from concourse.bass2jax import (
            _bass_exec_p,
            install_neuronx_cc_hook,
            partition_id_tensor,
        )

        try:
            from jax.experimental.shard_map import shard_map
        except ImportError:
            from jax.shard_map import shard_map

        install_neuronx_cc_hook()
        self.jax = jax
        partition_name = (
            nc.partition_id_tensor.name if nc.partition_id_tensor else None
        )
        in_names, out_names, out_avals, zero_outs = [], [], [], []
        for alloc in nc.m.functions[0].allocations:
            if not isinstance(alloc, mybir.MemoryLocationSet):
                continue
            name = alloc.memorylocations[0].name
            if alloc.kind == "ExternalInput":
                if name != partition_name:
                    in_names.append(name)
            elif alloc.kind == "ExternalOutput":
                shape = tuple(alloc.tensor_shape)
                dtype = mybir.dt.np(alloc.dtype)
                out_names.append(name)
                out_avals.append(jax.core.ShapedArray(shape, dtype))
                zero_outs.append(np.zeros((n_cores * shape[0], *shape[1:]), dtype))
        self.in_names = list(in_names)
        self.out_names = out_names
        self.out_avals = out_avals
        self.zero_outs = zero_outs
        self.n_cores = n_cores
        n_params = len(in_names)
        n_outs = len(out_names)
        all_in = in_names + out_names + ([partition_name] if partition_name else [])

        def _body(*args):
            operands = list(args)
            if partition_name is not None:
                operands.append(partition_id_tensor())
            return tuple(
                _bass_exec_p.bind(
                    *operands,
                    out_avals=tuple(out_avals),
                    in_names=tuple(all_in),
                    out_names=tuple(out_names),
                    lowering_input_output_aliases=(),
                    sim_require_finite=True,
                    sim_require_nnan=True,
                    nc=nc,
                )
            )

        devices = jax.devices()[:n_cores]
        mesh = Mesh(np.asarray(devices), ("core",))
        in_specs = (PartitionSpec("core"),) * (n_params + n_outs)
        out_specs = (PartitionSpec("core"),) * n_outs
        self.sharded = jax.jit(
            shard_map(
                _body, mesh=mesh, in_specs=in_specs, out_specs=out_specs,
                check_rep=False,
            ),
            donate_argnums=tuple(range(n_params, n_params + n_outs)),
            keep_unused=True,
        )

    def __call__(self, concat_in):
        """concat_in: dict name -> (n_cores*shape0, ...) array (numpy or
        pre-placed jax array). Returns list of per-core dicts of outputs."""
        args = [concat_in[n] for n in self.in_names]
        zeros = [np.zeros_like(z) for z in self.zero_outs]
        out_arrs = self.sharded(*args, *zeros)
        res = []
        for c in range(self.n_cores):
            res.append({
                name: np.asarray(out_arrs[i]).reshape(
                    self.n_cores, *self.out_avals[i].shape
                )[c]
                for i, name in enumerate(self.out_names)
            })
        return res


# ---------------------------------------------------------------------------
# Host orchestration
# ---------------------------------------------------------------------------


def kernel(**inputs):
    qe = np.asarray(inputs["query_embedding"], dtype=np.float32)
    keys = np.asarray(inputs["memory_keys"], dtype=np.float32)
    Wq = np.asarray(inputs["Wq"], dtype=np.float32)
    bq = np.asarray(inputs["bq"], dtype=np.float32)
    Wm = np.asarray(inputs["Wm"], dtype=np.float32)
    bm = np.asarray(inputs["bm"], dtype=np.float32)
    gam = np.asarray(inputs["ln_gamma"], dtype=np.float32)
    bet = np.asarray(inputs["ln_beta"], dtype=np.float32)
    Wc = np.asarray(inputs["Wc"], dtype=np.float32)
    bc_ = np.asarray(inputs["bc"], dtype=np.float32)
    k = int(inputs["k"])
    assert k == K and qe.shape == (B, D) and keys.shape == (N, D)

    import jax
    from jax.sharding import Mesh, NamedSharding, PartitionSpec

    # ---- phase 1 ----
    if "r1" not in _cache:
        _cache["r1"] = _SpmdRunner(_build_phase1())
    r1 = _cache["r1"]

    devices = jax.devices()[:NCORES]
    mesh = Mesh(np.asarray(devices), ("core",))
    csh = NamedSharding(mesh, PartitionSpec("core"))
    mn = np.sqrt(np.einsum("nd,nd->n", keys, keys, dtype=np.float64))
    mn32 = mn.astype(np.float32)
    q = np.maximum(qe, 0.0)
    qT_8 = np.ascontiguousarray(q.T).astype(NP8)            # [D, B]

    # normalized keys (prescaled), transposed, fp8, slab-permuted, sharded
    if "perm" not in _cache:
        _cache["perm"] = _p1_colperm()
    perm = _cache["perm"]
    parts = []
    for c in range(NCORES):
        sl = slice(c * SH, (c + 1) * SH)
        shard = (keys[sl].T * (KSCALE / mn32[sl])[None, :]).astype(NP8)
        parts.append(jax.device_put(shard[:, perm], devices[c]))
    keysTn_dev = jax.make_array_from_single_device_arrays(
        (NCORES * D, SH), csh, parts
    )

    res1 = r1({
        "qT": np.broadcast_to(qT_8, (NCORES, D, B)).reshape(NCORES * D, B),
        "keysTn": keysTn_dev,
    })

    # block maxima per query: [B, NCORES*NBLK]
    bmax = np.empty((B, NCORES * NBLK), np.float32)
    for c in range(NCORES):
        bm_c = res1[c]["bmax"].astype(np.float32)   # [2, 128, NBLK]
        bmax[:128, c * NBLK:(c + 1) * NBLK] = bm_c[0]
        bmax[128:, c * NBLK:(c + 1) * NBLK] = bm_c[1]
    bmax *= 1.0 / KSCALE

    # host merge: exact top-32 via margin-rescore of top-R blocks
    part = np.argpartition(-bmax, RTOP - 1, axis=1)[:, :RTOP]   # [B, R] blocks
    pv = np.take_along_axis(bmax, part, axis=1)
    T32 = -np.partition(-pv, K - 1, axis=1)[:, K - 1]           # 32nd block max
    # candidate keys of the R blocks
    cand = (part[:, :, None] * BLK + np.arange(BLK)[None, None, :]).reshape(
        B, RTOP * BLK
    )                                                           # [B, R*BLK]
    q64 = q.astype(np.float64)
    ck = keys[cand.reshape(-1)].reshape(B, RTOP * BLK, D)       # gather rows
    vex = np.einsum("bd,bkd->bk", q64, ck.astype(np.float64))
    vex /= mn[cand]
    sel = np.argpartition(-vex, K - 1, axis=1)[:, :K]
    top_idx = np.take_along_axis(cand, sel, axis=1)             # [B, K]

    # safety net: if > R blocks could clear the margin for some query,
    # rescore that query against the whole bank exactly.
    risky = np.where((bmax >= (T32 - MARGIN)[:, None]).sum(axis=1) > RTOP)[0]
    for b in risky:
        v_all = (keys.astype(np.float64) @ q64[b]) / mn
        top_idx[b] = np.argsort(-v_all, kind="stable")[:K]

    # ---- phase 2 ----
    if "r2" not in _cache:
        _cache["r2"] = _SpmdRunner(_build_phase2())
    r2 = _cache["r2"]
    qT_bf = np.ascontiguousarray(q.T).astype(BF)            # [D, B]
    mkT_cc = np.empty((NCORES, D, NK), BF)
    qT_cc = np.empty((NCORES, D, BQ), BF)
    for c in range(NCORES):
        flat = top_idx[c * BQ:(c + 1) * BQ].reshape(NK)
        mkT_cc[c] = keys[flat].T.astype(BF)                 # exact key rows
        qT_cc[c] = qT_bf[:, c * BQ:(c + 1) * BQ]

    def _rep(a):
        a = np.ascontiguousarray(a)
        return np.broadcast_to(a, (NCORES,) + a.shape).reshape(
            NCORES * a.shape[0], *a.shape[1:]
        )

    gbT = np.stack([gam, bet, -gam]).astype(np.float32)
    res2 = r2({
        "qTc": qT_cc.reshape(NCORES * D, BQ),
        "mkT": mkT_cc.reshape(NCORES * D, NK),
        "Wq": _rep(Wq.astype(BF)), "Wm": _rep(Wm.astype(BF)),
        "Wc": _rep(Wc.astype(BF)),
        "bq": _rep(bq), "bm": _rep(bm), "gbT": _rep(gbT), "bc_": _rep(bc_),
    })

    out = np.concatenate([res2[c]["out"] for c in range(NCORES)], axis=0)
    return out.astype(np.float32)


# revision 54
# speedup vs baseline: 1.0153x; 1.0153x over previous
"""Trainium2 Bass kernel for nn_MA_73478300500338 (retrieval_knn).

Pipeline (reference semantics):
  q = relu(query_embedding)                      [B, D]
  sim = cos(q, memory_keys); idx = top_k(sim, 32)
  mk = memory_keys[idx]
  qt = relu(q @ Wq + bq); mt = relu(mk @ Wm + bm)
  attended = sum_j mt[:, j, :]   (softmax over size-1 axis == 1)
  ma = LN(attended + qt) * gamma + beta
  out = [q, ma] @ Wc + bc                        [B, C]

Distribution (8 NeuronCores):
  Phase 1 (bf16 PE): memory bank sharded 8x (12500 rows/core). Each core
    computes bf16 dot products q . (k/|k|) for its shard (fp32 accumulate)
    and emits the max over each 10-wide block ("block-maxima", fp32) via DVE
    tensor_reduce straight out of PSUM. No top-k / index pass on device.
  Host: merges 8x2500 block maxima per query, takes the top R=64 blocks
    (provably a superset of every block containing a true top-32 key, given
    |bf16 dot err| <= eps; measured max err 1.0e-2 on this dataset, margin
    2*eps = 3e-2 used, max blocks needed 46), rescores those blocks exactly
    in fp64 and picks the exact global top-32. Exactness argument: the 32nd
    largest block-max T satisfies v32 >= T - eps (32 distinct keys sit at
    their block maxima), and every member's block-max >= v32 - eps, so all
    member blocks lie in {bmax >= T - 2*eps}. A count check + exact full
    fallback guards the fixed R.
  Phase 2 (bf16 PE): queries sharded 8x (32/core). Attention MLP, transpose-
    free layernorm (cross-partition moments via ones-matmul, gamma/beta
    applied through a rank-2 broadcast matmul), output projection.
"""

import os
import sys
import json

import numpy as np
import ml_dtypes

os.environ.setdefault("MYCRO_LOCAL_CACHE", "1")
if "/opt/trn_rl_repo" not in sys.path:
    sys.path.insert(0, "/opt/trn_rl_repo")

try:
    import jax as _jax
    _jax.config.update("jax_compilation_cache_dir", "/tmp/jax_cache_nn_ma")
    _jax.config.update("jax_persistent_cache_min_entry_size_bytes", -1)
    _jax.config.update("jax_persistent_cache_min_compile_time_secs", 0.5)
except Exception:
    pass

import bass_rust
import concourse.bass as bass
import concourse.bacc as bacc
import concourse.mybir as mybir
import concourse.tile as tile
from concourse.vector_clock import ScopedClock

# ---------------------------------------------------------------------------
# Workaround: this walrus build supports a single sync-wait per CTRL
# instruction, but Tile's stock tail drain carries one wait per busy
# processor. Split them into standalone single-wait instructions.
# ---------------------------------------------------------------------------


def _patched_drain_and_barrier(self, tick_clock, wait_clock):
    nc = self.nc
    with nc.discard():
        probe = nc.sync.drain()
        wait_clock.add_sem_waits(
            probe.ins, ScopedClock({None: tick_clock.global_clock})
        )
        j = json.loads(nc.instruction_to_json(probe.ins))
    waits = (j.get("sync_info") or {}).get("on_wait") or []
    for w in waits:
        sem = bass_rust.SemaphoreHandle(w["ant_name"], w["id"])
        assert w["wait_mode"] == "sem-ge-imm", w
        nc.sync.wait_ge(sem, w["wait_value"])
    nc.sync.drain()
    nc.all_engine_barrier()
    popped = nc._tile_sem_poison_stack.pop()
    assert popped is self._sem_poison
    nc.clear_and_free_semaphores(list(self.sems.allocated().values()))
    nc.all_engine_barrier()


tile.TileContext._drain_and_barrier = _patched_drain_and_barrier

# ---------------------------------------------------------------------------
# Problem shapes (hardcoded per spec)
# ---------------------------------------------------------------------------
B, N, D = 256, 100000, 512
AU, C, K = 256, 100, 32
NCORES = 8
SH = N // NCORES          # 12500 keys per core
W = 500                   # dot-product window (one PSUM-bank-pair round)
NW = SH // W              # 25 windows per core
BLK = 5                   # block-max granularity
WB = W // BLK             # 100 blocks per window
NBLK = SH // BLK          # 2500 blocks per core
DC = D // 128             # 4 contraction chunks
KSCALE = 32.0             # fp8 prescale on normalized keys (subnormal dodge)
EPS_LN = 1e-5
# ~2x the measured max |fp8 dot - exact| (0.168) incl. bf16 max rounding
MARGIN = 3.5e-1
RTOP = 320                # blocks rescored per query (max needed ~240)

F32 = mybir.dt.float32
BF16 = mybir.dt.bfloat16
F8 = mybir.dt.float8e4

BF = ml_dtypes.bfloat16
NP8 = mybir.dt.np(F8)

# phase-1 reduce paths: P1_ASET windows use DVE tensor_reduce on the
# original column order; all others are host-permuted to block-major and
# max-accumulated via 2x-mode bf16 tensor_max (Act copy + DVE slabs).
P1_ASET = frozenset({1, 6, 11, 16, 18, 21, 24})
P1_RUNS = [[0], [2, 3, 4, 5], [7, 8, 9, 10], [12, 13, 14, 15],
           [17], [19, 20], [22, 23]]


def _p1_colperm():
    """Device column p reads host column perm[p]; slab windows go
    block-major: device pos j*WB+nb <- original col nb*BLK+j."""
    perm = np.arange(SH).reshape(NW, W)
    slab = np.arange(W).reshape(WB, BLK).T.reshape(W)
    for w in range(NW):
        if w not in P1_ASET:
            perm[w] = w * W + slab
    return perm.reshape(SH)

_cache = {}


# ---------------------------------------------------------------------------
# Phase 1: bf16 dots + 10-wide block maxima
# ---------------------------------------------------------------------------


def _build_phase1():
    nc = bacc.Bacc()
    qT = nc.dram_tensor("qT", [D, B], F8, kind="ExternalInput")
    keysTn = nc.dram_tensor("keysTn", [D, SH], F8, kind="ExternalInput")
    bmax = nc.dram_tensor("bmax", [2, 128, NBLK], BF16, kind="ExternalOutput")

    # window groups per DMA: small at the front for a fast pipeline start
    GROUPS = [1, 2, 2, 4, 4, 4, 4, 4]
    assert sum(GROUPS) == NW
    OSPLITS = [11, 16, 21, 25]  # chunk boundaries aligned to slab runs

    with tile.TileContext(nc) as tc:
        with (
            tc.tile_pool(name="persist", bufs=1) as persist,
            tc.tile_pool(name="keys", bufs=3) as keysp,
            tc.tile_pool(name="cw", bufs=2) as cwp,
            tc.tile_pool(name="psum", bufs=4, space="PSUM") as psump,
        ):
            # PE warm-up: the pstate ramp needs ~3us of continuous busy to
            # reach full clock; burn the DMA lead-in on dummy matmuls.
            # The scratch PSUM comes from the ps pool (recycled later).
            scr = persist.tile([128, 256], F8)
            nc.gpsimd.memset(scr, 0.25)
            # pin the Act function table (Copy) during the lead-in too
            pint = persist.tile([1, 2], F32)
            nc.vector.memset(pint, 0.0)
            nc.scalar.copy(out=pint[:, 1:2], in_=pint[:, 0:1])
            wps = psump.tile([128, 2, 512], F32, tag="ps")
            for i in range(10):
                nc.tensor.matmul(
                    wps[0:16, 0, 0:256], scr[:, 0:16], scr,
                    start=True, stop=True,
                )

            qr = persist.tile([128, DC, B], F8)
            nc.sync.dma_start(
                out=qr,
                in_=bass.AP(qT, 0, [[B, 128], [128 * B, DC], [1, B]]),
            )
            bs = persist.tile([128, 2, NBLK], BF16)

            emitted = set()          # windows whose bs write has been emitted
            flushed = [False] * len(OSPLITS)
            pending = []             # deferred slab-group emissions

            def emit_group(cw_w0, rl, cw):
                bs_g = bs[
                    :, :, cw_w0 * WB:(cw_w0 + rl) * WB
                ].rearrange("p b (ws nb) -> p b ws nb", nb=WB)
                nc.vector.tensor_max(
                    out=bs_g,
                    in0=cw[:, :, :, 0:WB],
                    in1=cw[:, :, :, WB:2 * WB],
                )
                for j in range(2, BLK):
                    nc.vector.tensor_max(
                        out=bs_g, in0=bs_g,
                        in1=cw[:, :, :, j * WB:(j + 1) * WB],
                    )
                emitted.update(range(cw_w0, cw_w0 + rl))

            def try_flush():
                for i, osp in enumerate(OSPLITS):
                    if not flushed[i] and all(x in emitted for x in range(osp)):
                        flushed[i] = True
                        lo = (OSPLITS[i - 1] if i else 0) * WB
                        nc.gpsimd.dma_start(
                            out=bass.AP(
                                bmax, lo,
                                [[NBLK, 128], [128 * NBLK, 2],
                                 [1, osp * WB - lo]],
                            ),
                            in_=bs[:, :, lo:osp * WB],
                        )

            w0 = 0
            for gw in GROUPS:
                kt = keysp.tile([128, DC, gw * W], F8, tag="kt")
                nc.sync.dma_start(
                    out=kt,
                    in_=bass.AP(
                        keysTn,
                        w0 * W,
                        [[SH, 128], [128 * SH, DC], [1, gw * W]],
                    ),
                )
                for wi in range(gw):
                    w = w0 + wi
                    ps = psump.tile([128, 2, 512], F32, tag="ps")
                    for bc in range(2):
                        for c2 in range(2):
                            # fp8 DoubleRow: one pass contracts 256 rows
                            nc.tensor.matmul(
                                ps[:, bc, 0:W],
                                qr[:, 2 * c2:2 * c2 + 2,
                                   bc * 128:(bc + 1) * 128],
                                kt[:, 2 * c2:2 * c2 + 2,
                                   wi * W:(wi + 1) * W],
                                start=(c2 == 0),
                                stop=(c2 == 1),
                                perf_mode=mybir.MatmulPerfMode.DoubleRow,
                            )
                    if w in P1_ASET:
                        # direct path: DVE block-max straight from PSUM;
                        # emit before any pending slab group (it is ready
                        # earlier, and DVE executes in-order)
                        nc.vector.tensor_reduce(
                            out=bs[:, :, w * WB:(w + 1) * WB],
                            in_=ps[:, :, 0:W].rearrange(
                                "p b (nb k) -> p b nb k", k=BLK
                            ),
                            axis=mybir.AxisListType.X,
                            op=mybir.AluOpType.max,
                        )
                        emitted.add(w)
                        while pending:
                            emit_group(*pending.pop(0))
                    else:
                        # slab path (host permuted these windows to block-
                        # major): Act copies PSUM->SBUF bf16 per window; DVE
                        # max-accumulates the run's 5 packed slabs in 2x
                        # mode, deferred past the next direct reduce
                        run = next(r for r in P1_RUNS if w in r)
                        slot = run.index(w)
                        rl = len(run)
                        if slot == 0:
                            cw = cwp.tile([128, 2, rl, 512], BF16, tag=f"cw{rl}")
                            cw_w0 = w
                        nc.scalar.copy(
                            out=cw[:, :, slot, 0:W], in_=ps[:, :, 0:W]
                        )
                        if slot == rl - 1:
                            pending.append((cw_w0, rl, cw))
                    try_flush()
                w0 += gw
            while pending:
                emit_group(*pending.pop(0))
            try_flush()
    nc.finalize()
    return nc


# ---------------------------------------------------------------------------
# Phase 2: attention MLP + LN + output projection (32 queries per core)
# ---------------------------------------------------------------------------
BQ = B // NCORES          # 32 queries per core
NK = BQ * K               # 1024 gathered key columns per core
AC = AU // 128            # 2 au chunks
WCC = (D + AU) // 128     # 6 Wc contraction chunks


def _build_phase2():
    nc = bacc.Bacc()
    qTc = nc.dram_tensor("qTc", [D, BQ], BF16, kind="ExternalInput")
    mkT = nc.dram_tensor("mkT", [D, NK], BF16, kind="ExternalInput")
    Wq = nc.dram_tensor("Wq", [D, AU], BF16, kind="ExternalInput")
    Wm = nc.dram_tensor("Wm", [D, AU], BF16, kind="ExternalInput")
    Wc = nc.dram_tensor("Wc", [D + AU, C], BF16, kind="ExternalInput")
    bq = nc.dram_tensor("bq", [AU], F32, kind="ExternalInput")
    bm = nc.dram_tensor("bm", [AU], F32, kind="ExternalInput")
    # rows: gamma, beta, -gamma
    gbT = nc.dram_tensor("gbT", [3, AU], F32, kind="ExternalInput")
    bc_ = nc.dram_tensor("bc_", [C], F32, kind="ExternalInput")
    out = nc.dram_tensor("out", [BQ, C], F32, kind="ExternalOutput")

    with tile.TileContext(nc) as tc:
        with (
            tc.tile_pool(name="p", bufs=1) as pool,
            tc.tile_pool(name="psum", bufs=2, space="PSUM") as psump,
            tc.tile_pool(name="psum1", bufs=1, space="PSUM") as psump1,
            tc.tile_pool(name="psumo", bufs=1, space="PSUM") as psumo,
            tc.tile_pool(name="warm", bufs=1, space="PSUM") as warmp,
        ):
            # PE warm-up during the DMA lead-in (pstate ramp)
            scr = pool.tile([128, 256], BF16)
            nc.gpsimd.memset(scr, 0.25)
            wps = warmp.tile([16, 256], F32)
            for i in range(10):
                nc.tensor.matmul(
                    wps, scr[:, 0:16], scr, start=True, stop=True,
                )
            # pin the activation table to the set holding relu+square+sqrt
            # +copy so no mid-kernel table reload is needed
            sqscr = pool.tile([1, 1], F32)
            nc.vector.memset(sqscr, 1.0)
            nc.scalar.sqrt(out=sqscr, in_=sqscr)

            # ---- loads: tiny constants via the Pool queue (SWDGE; keeps
            # the SP/HWDGE path clear for the big loads) ----
            bmc = pool.tile([128, AC], F32)
            nc.gpsimd.dma_start(out=bmc, in_=bass.AP(bm, 0, [[1, 128], [128, AC]]))
            bqc = pool.tile([128, AC], F32)
            nc.gpsimd.dma_start(out=bqc, in_=bass.AP(bq, 0, [[1, 128], [128, AC]]))

            wm = pool.tile([128, DC, AU], BF16)
            nc.sync.dma_start(
                out=wm, in_=bass.AP(Wm, 0, [[AU, 128], [128 * AU, DC], [1, AU]])
            )
            # mk h0 split by contraction pairs so the first matmul can
            # start after ~1/4 of the mk bytes have landed
            mk = pool.tile([128, DC, NK], BF16)
            for c2 in range(2):
                nc.sync.dma_start(
                    out=mk[:, 2 * c2:2 * c2 + 2, 0:512],
                    in_=bass.AP(
                        mkT, 2 * c2 * 128 * NK,
                        [[NK, 128], [128 * NK, 2], [1, 512]]
                    ),
                )
            nc.sync.dma_start(
                out=mk[:, :, 512:1024],
                in_=bass.AP(
                    mkT, 512, [[NK, 128], [128 * NK, DC], [1, 512]]
                ),
            )
            qr = pool.tile([128, DC, BQ], BF16)
            nc.sync.dma_start(
                out=qr, in_=bass.AP(qTc, 0, [[BQ, 128], [128 * BQ, DC], [1, BQ]])
            )
            wq = pool.tile([128, DC, AU], BF16)
            nc.sync.dma_start(
                out=wq, in_=bass.AP(Wq, 0, [[AU, 128], [128 * AU, DC], [1, AU]])
            )
            wc = pool.tile([128, WCC, C], BF16)
            nc.sync.dma_start(
                out=wc, in_=bass.AP(Wc, 0, [[C, 128], [128 * C, WCC], [1, C]])
            )
            # gamma/beta/-gamma as partition-0 rows (needed late; Pool queue)
            gb = pool.tile([1, 3, AC, 128], F32)
            nc.gpsimd.dma_start(
                out=gb, in_=bass.AP(gbT, 0, [[0, 1], [AU, 3], [128, AC], [1, 128]])
            )
            # bc broadcast rows [BQ, C]
            bcrow = pool.tile([BQ, C], F32)
            nc.gpsimd.dma_start(out=bcrow, in_=bass.AP(bc_, 0, [[0, BQ], [1, C]]))


            onesc = pool.tile([128, 1], F32)
            nc.vector.memset(onesc, 1.0)

            # ---- mtT = relu(Wm^T mk + bm): [AU, NK] f32 ----
            mtT = pool.tile([128, AC, NK], F32)
            for a in range(AC):
                for h in range(2):
                    ps = psump.tile([128, 512], F32, tag="ps")
                    for c in range(DC):
                        nc.tensor.matmul(
                            ps,
                            wm[:, c, a * 128:(a + 1) * 128],
                            mk[:, c, h * 512:(h + 1) * 512],
                            start=(c == 0),
                            stop=(c == DC - 1),
                        )
                    nc.scalar.activation(
                        out=mtT[:, a, h * 512:(h + 1) * 512],
                        in_=ps,
                        func=mybir.ActivationFunctionType.Relu,
                        bias=bmc[:, a:a + 1],
                        scale=1.0,
                    )

            # ---- xT = attendedT + qtT: [AU, BQ] f32 ----
            # attended reduce split per mk half: cols h*512.. hold queries
            # h*16.., so each half-reduce lands in its own query slice
            xT = pool.tile([128, AC, BQ], F32)
            attT = pool.tile([128, AC, BQ], F32)
            for a in range(AC):
                for h in range(2):
                    nc.vector.tensor_reduce(
                        out=attT[:, a, h * 16:(h + 1) * 16],
                        in_=mtT[:, a, h * 512:(h + 1) * 512].rearrange(
                            "p (b j) -> p b j", j=K
                        ),
                        axis=mybir.AxisListType.X,
                        op=mybir.AluOpType.add,
                    )
                ps = psump.tile([128, BQ], F32, tag="psq")
                for c in range(DC):
                    nc.tensor.matmul(
                        ps,
                        wq[:, c, a * 128:(a + 1) * 128],
                        qr[:, c, :],
                        start=(c == 0),
                        stop=(c == DC - 1),
                    )
                qt_a = pool.tile([128, BQ], F32, tag=f"qt{a}")
                nc.scalar.activation(
                    out=qt_a,
                    in_=ps,
                    func=mybir.ActivationFunctionType.Relu,
                    bias=bqc[:, a:a + 1],
                    scale=1.0,
                )
                nc.vector.tensor_add(out=xT[:, a, :], in0=attT[:, a, :], in1=qt_a)

            # ---- layernorm, transpose-free ----
            # moments via ones-matmul: [1, 2, BQ] = [sum(x); sum(x^2)]
            xsq = pool.tile([128, AC, BQ], F32)
            nc.scalar.activation(
                out=xsq, in_=xT, func=mybir.ActivationFunctionType.Square
            )
            psm = psump1.tile([1, 2, BQ], F32, tag="psm")
            for a in range(AC):
                nc.tensor.matmul(
                    psm[:, 0, :], onesc, xT[:, a, :],
                    start=(a == 0), stop=(a == AC - 1),
                )
            for a in range(AC):
                nc.tensor.matmul(
                    psm[:, 1, :], onesc, xsq[:, a, :],
                    start=(a == 0), stop=(a == AC - 1),
                )
            # rows [1, 8, BQ]: 0=mu 1=ex2 2=mu^2 3=var 4=std 5=rstd 6=mu*rstd
            rows = pool.tile([1, 8, BQ], F32)
            nc.scalar.mul(out=rows[:, 0:2, :], in_=psm, mul=1.0 / AU)
            nc.vector.tensor_mul(
                out=rows[:, 2, :], in0=rows[:, 0, :], in1=rows[:, 0, :]
            )
            nc.vector.tensor_sub(
                out=rows[:, 3, :], in0=rows[:, 1, :], in1=rows[:, 2, :]
            )
            eps = pool.tile([1, 1], F32)
            nc.vector.memset(eps, EPS_LN)
            nc.scalar.activation(
                out=rows[:, 4, :], in_=rows[:, 3, :],
                func=mybir.ActivationFunctionType.Sqrt,
                bias=eps, scale=1.0,
            )
            nc.vector.reciprocal(out=rows[:, 5, :], in_=rows[:, 4, :])
            nc.vector.tensor_mul(
                out=rows[:, 6, :], in0=rows[:, 0, :], in1=rows[:, 5, :]
            )
            ones_row = pool.tile([1, BQ], F32)
            nc.vector.memset(ones_row, 1.0)

            # C1 = gamma x rstd  (PSUM [128, AC, BQ]);
            # C0 = (-gamma) x (mu*rstd) + beta x ones
            psC = psump1.tile([128, 2, AC, BQ], F32, tag="psC")
            for a in range(AC):
                nc.tensor.matmul(
                    psC[:, 0, a, :], gb[:, 0, a, :], rows[:, 5, :],
                    start=True, stop=True,
                )
                nc.tensor.matmul(
                    psC[:, 1, a, :], gb[:, 2, a, :], rows[:, 6, :],
                    start=True, stop=False,
                )
                nc.tensor.matmul(
                    psC[:, 1, a, :], gb[:, 1, a, :], ones_row,
                    start=False, stop=True,
                )
            # maT = xT * C1 + C0, bf16 (both a-chunks in one op each)
            maT = pool.tile([128, AC, BQ], BF16)
            tmp = pool.tile([128, AC, BQ], F32)
            nc.vector.tensor_mul(out=tmp, in0=xT, in1=psC[:, 0, :, :])
            nc.vector.tensor_add(out=maT, in0=tmp, in1=psC[:, 1, :, :])

            # ---- out = [q, ma] @ Wc + bc ----
            pso = psumo.tile([BQ, C], F32, tag="pso")
            for c in range(DC):
                nc.tensor.matmul(
                    pso, qr[:, c, :], wc[:, c, :],
                    start=(c == 0), stop=False,
                )
            for a in range(AC):
                nc.tensor.matmul(
                    pso, maT[:, a, :], wc[:, DC + a, :],
                    start=False, stop=(a == AC - 1),
                )
            ot = pool.tile([BQ, C], F32)
            nc.vector.tensor_add(out=ot, in0=bcrow, in1=pso)
            nc.sync.dma_start(out=out[:, :], in_=ot)
    nc.finalize()
    return nc


# ---------------------------------------------------------------------------
# SPMD runner with a persistent jitted executable
# ---------------------------------------------------------------------------


class _SpmdRunner:
    def __init__(self, nc, n_cores=NCORES):
        import jax
        from jax.sharding import Mesh, PartitionSpec
        from concourse import bass2jax
        from concourse.bass2jax import (
            _bass_exec_p,
            install_neuronx_cc_hook,
            partition_id_tensor,
        )

        try:
            from jax.experimental.shard_map import shard_map
        except ImportError:
            from jax.shard_map import shard_map

        install_neuronx_cc_hook()
        self.jax = jax
        partition_name = (
            nc.partition_id_tensor.name if nc.partition_id_tensor else None
        )
        in_names, out_names, out_avals, zero_outs = [], [], [], []
        for alloc in nc.m.functions[0].allocations:
            if not isinstance(alloc, mybir.MemoryLocationSet):
                continue
            name = alloc.memorylocations[0].name
            if alloc.kind == "ExternalInput":
                if name != partition_name:
                    in_names.append(name)
            elif alloc.kind == "ExternalOutput":
                shape = tuple(alloc.tensor_shape)
                dtype = mybir.dt.np(alloc.dtype)
                out_names.append(name)
                out_avals.append(jax.core.ShapedArray(shape, dtype))
                zero_outs.append(np.zeros((n_cores * shape[0], *shape[1:]), dtype))
        self.in_names = list(in_names)
        self.out_names = out_names
        self.out_avals = out_avals
        self.zero_outs = zero_outs
        self.n_cores = n_cores
        n_params = len(in_names)
        n_outs = len(out_names)
        all_in = in_names + out_names + ([partition_name] if partition_name else [])

        def _body(*args):
            operands = list(args)
            if partition_name is not None:
                operands.append(partition_id_tensor())
            return tuple(
                _bass_exec_p.bind(
                    *operands,
                    out_avals=tuple(out_avals),
                    in_names=tuple(all_in),
                    out_names=tuple(out_names),
                    lowering_input_output_aliases=(),
                    sim_require_finite=True,
                    sim_require_nnan=True,
                    nc=nc,
                )
            )

        devices = jax.devices()[:n_cores]
        mesh = Mesh(np.asarray(devices), ("core",))
        in_specs = (PartitionSpec("core"),) * (n_params + n_outs)
        out_specs = (PartitionSpec("core"),) * n_outs
        self.sharded = jax.jit(
            shard_map(
                _body, mesh=mesh, in_specs=in_specs, out_specs=out_specs,
                check_rep=False,
            ),
            donate_argnums=tuple(range(n_params, n_params + n_outs)),
            keep_unused=True,
        )

    def __call__(self, concat_in):
        """concat_in: dict name -> (n_cores*shape0, ...) array (numpy or
        pre-placed jax array). Returns list of per-core dicts of outputs."""
        args = [concat_in[n] for n in self.in_names]
        zeros = [np.zeros_like(z) for z in self.zero_outs]
        out_arrs = self.sharded(*args, *zeros)
        res = []
        for c in range(self.n_cores):
            res.append({
                name: np.asarray(out_arrs[i]).reshape(
                    self.n_cores, *self.out_avals[i].shape
                )[c]
                for i, name in enumerate(self.out_names)
            })
        return res


# ---------------------------------------------------------------------------
# Host orchestration
# ---------------------------------------------------------------------------


def kernel(**inputs):
    qe = np.asarray(inputs["query_embedding"], dtype=np.float32)
    keys = np.asarray(inputs["memory_keys"], dtype=np.float32)
    Wq = np.asarray(inputs["Wq"], dtype=np.float32)
    bq = np.asarray(inputs["bq"], dtype=np.float32)
    Wm = np.asarray(inputs["Wm"], dtype=np.float32)
    bm = np.asarray(inputs["bm"], dtype=np.float32)
    gam = np.asarray(inputs["ln_gamma"], dtype=np.float32)
    bet = np.asarray(inputs["ln_beta"], dtype=np.float32)
    Wc = np.asarray(inputs["Wc"], dtype=np.float32)
    bc_ = np.asarray(inputs["bc"], dtype=np.float32)
    k = int(inputs["k"])
    assert k == K and qe.shape == (B, D) and keys.shape == (N, D)

    import jax
    from jax.sharding import Mesh, NamedSharding, PartitionSpec

    # ---- phase 1 ----
    if "r1" not in _cache:
        _cache["r1"] = _SpmdRunner(_build_phase1())
    r1 = _cache["r1"]

    devices = jax.devices()[:NCORES]
    mesh = Mesh(np.asarray(devices), ("core",))
    csh = NamedSharding(mesh, PartitionSpec("core"))
    mn = np.sqrt(np.einsum("nd,nd->n", keys, keys, dtype=np.float64))
    mn32 = mn.astype(np.float32)
    q = np.maximum(qe, 0.0)
    qT_8 = np.ascontiguousarray(q.T).astype(NP8)            # [D, B]

    # normalized keys (prescaled), transposed, fp8, slab-permuted, sharded
    if "perm" not in _cache:
        _cache["perm"] = _p1_colperm()
    perm = _cache["perm"]
    parts = []
    for c in range(NCORES):
        sl = slice(c * SH, (c + 1) * SH)
        shard = (keys[sl].T * (KSCALE / mn32[sl])[None, :]).astype(NP8)
        parts.append(jax.device_put(shard[:, perm], devices[c]))
    keysTn_dev = jax.make_array_from_single_device_arrays(
        (NCORES * D, SH), csh, parts
    )

    res1 = r1({
        "qT": np.broadcast_to(qT_8, (NCORES, D, B)).reshape(NCORES * D, B),
        "keysTn": keysTn_dev,
    })

    # block maxima per query: [B, NCORES*NBLK]
    bmax = np.empty((B, NCORES * NBLK), np.float32)
    for c in range(NCORES):
        bm_c = res1[c]["bmax"].astype(np.float32)   # [2, 128, NBLK]
        bmax[:128, c * NBLK:(c + 1) * NBLK] = bm_c[0]
        bmax[128:, c * NBLK:(c + 1) * NBLK] = bm_c[1]
    bmax *= 1.0 / KSCALE

    # host merge: exact top-32 via margin-rescore of top-R blocks
    part = np.argpartition(-bmax, RTOP - 1, axis=1)[:, :RTOP]   # [B, R] blocks
    pv = np.take_along_axis(bmax, part, axis=1)
    T32 = -np.partition(-pv, K - 1, axis=1)[:, K - 1]           # 32nd block max
    # candidate keys of the R blocks
    cand = (part[:, :, None] * BLK + np.arange(BLK)[None, None, :]).reshape(
        B, RTOP * BLK
    )                                                           # [B, R*BLK]
    q64 = q.astype(np.float64)
    ck = keys[cand.reshape(-1)].reshape(B, RTOP * BLK, D)       # gather rows
    vex = np.einsum("bd,bkd->bk", q64, ck.astype(np.float64))
    vex /= mn[cand]
    sel = np.argpartition(-vex, K - 1, axis=1)[:, :K]
    top_idx = np.take_along_axis(cand, sel, axis=1)             # [B, K]

    # safety net: if > R blocks could clear the margin for some query,
    # rescore that query against the whole bank exactly.
    risky = np.where((bmax >= (T32 - MARGIN)[:, None]).sum(axis=1) > RTOP)[0]
    for b in risky:
        v_all = (keys.astype(np.float64) @ q64[b]) / mn
        top_idx[b] = np.argsort(-v_all, kind="stable")[:K]

    # ---- phase 2 ----
    if "r2" not in _cache:
        _cache["r2"] = _SpmdRunner(_build_phase2())
    r2 = _cache["r2"]
    qT_bf = np.ascontiguousarray(q.T).astype(BF)            # [D, B]
    mkT_cc = np.empty((NCORES, D, NK), BF)
    qT_cc = np.empty((NCORES, D, BQ), BF)
    for c in range(NCORES):
        flat = top_idx[c * BQ:(c + 1) * BQ].reshape(NK)
        mkT_cc[c] = keys[flat].T.astype(BF)                 # exact key rows
        qT_cc[c] = qT_bf[:, c * BQ:(c + 1) * BQ]

    def _rep(a):
        a = np.ascontiguousarray(a)
        return np.broadcast_to(a, (NCORES,) + a.shape).reshape(
            NCORES * a.shape[0], *a.shape[1:]
        )

    gbT = np.stack([gam, bet, -gam]).astype(np.float32)
    res2 = r2({
        "qTc": qT_cc.reshape(NCORES * D, BQ),
        "mkT": mkT_cc.reshape(NCORES * D, NK),
        "Wq": _rep(Wq.astype(BF)), "Wm": _rep(Wm.astype(BF)),
        "Wc": _rep(Wc.astype(BF)),
        "bq": _rep(bq), "bm": _rep(bm), "gbT": _rep(gbT), "bc_": _rep(bc_),
    })

    out = np.concatenate([res2[c]["out"] for c in range(NCORES)], axis=0)
    return out.astype(np.float32)
